# revision 1
# baseline (speedup 1.0000x reference)
"""CenterCut2 Trainium2 kernel.

For each sample b: find argmax of power = sum_c x[b,c]^2 over the (D,H,W)
volume, then extract the 16x32x32 window centered on the peak with circular
wraparound (equivalent to reference's per-sample roll + center crop).

Sharding: pure data parallelism, 4 samples per core across 8 cores.

Per-core device program (samples s=0..3, volumes v=2s+c laid out [128, 8192]
with flat voxel index = p*8192 + f = dd*16384 + hh*128 + w):
  Per sample (pipelined so sample s's window extraction overlaps sample s+1's
  streaming):
  1. Stream both channel volumes; power = x0^2 (ACT) + x1^2 (ACT, in place)
     summed on DVE.
  2. vector.max + max_index give the per-partition argmax; partition_all_reduce
     max with a BIG-constant tie-break selects the global flat index (lowest
     flat index on exact ties, matching jnp.argmax); DVE integer ops decompose
     it into d, h0, w0, s_h and the two 32-row h-chunk ids c0/c1.
  3. One 64-descriptor dma_gather (16KB per descriptor) pulls, for each of the
     16 d-slices and 2 channels, the two 32-row h-chunks covering the
     h-window; rows land at partitions chunk*32 + c*16 + i (base 0).
  4. The two chunk halves are merged into a [32, 64, 160] tile doubled along w
     (merge copies split between ACT and DVE; partition-shifted halves on DVE),
     and a single register-offset (bass.ds) strided copy extracts the
     [32h x 32w] window. One [32, 1024] DMA writes the sample's output.
"""
import sys

sys.path.insert(0, "/opt/trn_rl_repo")

import numpy as np

import concourse.bass as bass
import concourse.bacc as bacc
import concourse.mybir as mybir
from concourse.tile import TileContext
from concourse.tile_rust import add_dep_helper
from concourse.bass_utils import run_bass_kernel_spmd
from concourse.bass_isa import ReduceOp

F32 = mybir.dt.float32
I32 = mybir.dt.int32
I16 = mybir.dt.int16
A = mybir.AluOpType
DVE = mybir.EngineType.DVE

N_CORES = 8
S_PER_CORE = 4          # samples per core
N_VOLS = 2 * S_PER_CORE # channel volumes per core
VOL = 64 * 128 * 128    # voxels per volume
FREE = VOL // 128       # 8192 free elements per partition
CHUNK = 4096            # streaming chunk (2 MiB per DMA)
BIG = float(1 << 21)

_cache = {}


def _build(loop_k=None):
    nc = bacc.Bacc("TRN2", target_bir_lowering=False, debug=False, num_devices=N_CORES)
    x = nc.dram_tensor("x", [N_VOLS, 128, FREE], F32, kind="ExternalInput")
    y = nc.dram_tensor("y", [128, 1024], F32, kind="ExternalOutput")

    iota_base_c = nc.inline_tensor(
        (np.arange(128, dtype=np.float32) * FREE).reshape(128, 1), name="iota_base"
    )
    iotaq_c = nc.inline_tensor(np.arange(16, dtype=np.int32).reshape(16, 1), name="iotaq")
    # gather source view: [2048 rows, 4096] — row = vol*256 + dd*4 + hchunk
    xrows = x.ap().rearrange("v p (a b) -> (v p a) b", a=2)

    with TileContext(nc) as tc:
        with (
            tc.tile_pool(name="xc", bufs=3) as xpool,
            tc.tile_pool(name="pw", bufs=3) as ppool,
            tc.tile_pool(name="sm", bufs=2) as spool,
            tc.tile_pool(name="ob", bufs=2) as opool,
            tc.tile_pool(name="big", bufs=1) as bpool,
        ):
          def body(_iv=None):
            base = bpool.tile([128, 1], F32, tag="base")
            nc.sync.dma_start(base[:, :], iota_base_c.ap()[:, :])
            iotaq = bpool.tile([16, 1], I32, tag="iotaq")
            nc.sync.dma_start(iotaq[:, :], iotaq_c.ap()[:, :])
            scal = bpool.tile([1, 64], I32, tag="scal")

            def ts(dst, src, s1, op0):
                return nc.vector.tensor_scalar(
                    out=dst, in0=src, scalar1=s1, scalar2=None, op0=op0
                )

            for s in range(S_PER_CORE):
                power = ppool.tile([128, FREE], F32, tag="pw")
                # stream both channels, build power map
                for k in range(FREE // CHUNK):
                    sl = slice(k * CHUNK, (k + 1) * CHUNK)
                    x0 = xpool.tile([128, CHUNK], F32, tag="xc")
                    nc.sync.dma_start(x0[:, :], x[2 * s, :, sl])
                    nc.scalar.square(power[:, sl], x0[:, :])
                    x1 = xpool.tile([128, CHUNK], F32, tag="xc")
                    nc.sync.dma_start(x1[:, :], x[2 * s + 1, :, sl])
                    nc.scalar.square(x1[:, :], x1[:, :])  # in place
                    nc.vector.tensor_add(power[:, sl], power[:, sl], x1[:, :])

                # per-partition top-1 value + index
                max8 = spool.tile([128, 8], F32, tag="max8")
                idx8 = spool.tile([128, 8], mybir.dt.uint32, tag="idx8")
                nc.vector.max(out=max8[:, :], in_=power[:, :])
                nc.vector.max_index(out=idx8[:, :], in_max=max8[:, :], in_values=power[:, :])

                # global argmax with lowest-flat tie-break
                flatf = spool.tile([128, 1], F32, tag="flatf")
                nc.vector.tensor_copy(flatf[:, :], idx8[:, 0:1])      # uint32 -> f32
                nc.vector.tensor_add(flatf[:, :], flatf[:, :], base[:, :])
                allmax = spool.tile([128, 1], F32, tag="allmax")
                nc.gpsimd.partition_all_reduce(allmax[:, :], max8[:, 0:1], 128, ReduceOp.max)
                eq = spool.tile([128, 1], F32, tag="eq")
                nc.vector.tensor_tensor(out=eq[:, :], in0=max8[:, 0:1], in1=allmax[:, :], op=A.is_equal)
                candneg = spool.tile([128, 1], F32, tag="candneg")
                nc.vector.scalar_tensor_tensor(
                    out=candneg[:, :], in0=eq[:, :], scalar=BIG, in1=flatf[:, :],
                    op0=A.mult, op1=A.subtract,
                )
                allcand = spool.tile([128, 1], F32, tag="allcand")
                nc.gpsimd.partition_all_reduce(allcand[:, :], candneg[:, :], 128, ReduceOp.max)

                # flat = BIG - allcand -> int32 scalar column block for sample s
                def C(j):
                    return scal[:, 16 * s + j : 16 * s + j + 1]

                flat32 = spool.tile([1, 1], F32, tag="flat32")
                nc.vector.tensor_scalar(
                    out=flat32[:, :], in0=allcand[0:1, 0:1], scalar1=BIG, scalar2=-1.0,
                    op0=A.subtract, op1=A.mult,
                )
                nc.vector.tensor_copy(C(0), flat32[:, :])             # f32 -> int32
                ts(C(1), C(0), 14, A.logical_shift_right)             # d
                ts(C(2), C(0), 7, A.logical_shift_right)
                ts(C(2), C(2), 127, A.bitwise_and)                    # h
                ts(C(3), C(0), 127, A.bitwise_and)                    # w
                ts(C(4), C(2), 112, A.add)
                ts(C(4), C(4), 127, A.bitwise_and)                    # h0
                ts(C(5), C(3), 112, A.add)
                w_w0 = ts(C(5), C(5), 127, A.bitwise_and)             # w0
                w_sh = ts(C(6), C(4), 31, A.bitwise_and)              # s_h
                ts(C(9), C(1), 56, A.add)                             # d + 56
                ts(C(10), C(4), 5, A.logical_shift_right)             # c0
                ts(C(11), C(4), 31, A.add)
                ts(C(11), C(11), 127, A.bitwise_and)
                ts(C(11), C(11), 5, A.logical_shift_right)            # c1

                # gather row indices for this sample: 64 idxs in wrapped
                # [16, 4] int16 layout; position n = chunk*32 + c*16 + i
                bc3 = spool.tile([16, 3], I32, tag="bc3")
                nc.gpsimd.partition_broadcast(bc3[:, :], scal[0:1, 16 * s + 9 : 16 * s + 12], channels=16)
                dterm = spool.tile([16, 1], I32, tag="dterm")
                nc.vector.tensor_tensor(out=dterm[:, :], in0=iotaq[:, :], in1=bc3[:, 0:1], op=A.add)
                ts(dterm[:, :], dterm[:, :], 63, A.bitwise_and)
                ts(dterm[:, :], dterm[:, :], 2, A.logical_shift_left)
                idx32 = spool.tile([16, 6], I32, tag="idx32")
                nc.vector.tensor_tensor(out=idx32[:, 4:5], in0=dterm[:, :], in1=bc3[:, 1:2], op=A.add)
                nc.vector.tensor_tensor(out=idx32[:, 5:6], in0=dterm[:, :], in1=bc3[:, 2:3], op=A.add)
                for t in range(4):
                    ts(idx32[:, t : t + 1], idx32[:, 4 + t // 2 : 5 + t // 2], (2 * s + t % 2) * 256, A.add)
                idx16 = spool.tile([16, 4], I16, tag="idx16")
                nc.vector.tensor_copy(idx16[:, :], idx32[:, 0:4])
                idxrep = spool.tile([128, 4], I16, tag="idxrep")
                for g in range(8):
                    nc.sync.dma_start(idxrep[16 * g : 16 * g + 16, :], idx16[:, :])

                # 64 x 16KB gather: all window rows for this sample
                G = ppool.tile([128, 4096], F32, tag="pw")
                nc.gpsimd.dma_gather(
                    out_ap=G[:, :].rearrange("p (a b) -> p a b", a=1),
                    in_ap=xrows,
                    idxs_ap=idxrep[:, :],
                    num_idxs=64,
                    num_idxs_reg=64,
                    elem_size=4096,
                )

                # merge chunk halves into partition-base-0 doubled tile
                Ds = ppool.tile([32, 10240], F32, tag="pw")
                d3 = Ds[:, :].rearrange("p (a b) -> p a b", b=160)
                g3a = G[0:32, :].rearrange("p (a b) -> p a b", b=128)
                g3b = G[32:64, :].rearrange("p (a b) -> p a b", b=128)
                nc.scalar.copy(d3[:, 0:32, 0:128], g3a[:, :, :])
                nc.vector.tensor_copy(d3[:, 32:64, 0:128], g3b[:, :, :])
                nc.scalar.copy(d3[:, 0:32, 128:160], g3a[:, :, 0:32])
                nc.vector.tensor_copy(d3[:, 32:64, 128:160], g3b[:, :, 0:32])

                # dynamic window selection
                li_sh, (sh_val,) = nc.values_load_multi_w_load_instructions(
                    scal[0:1, 16 * s + 6 : 16 * s + 7], engines=(DVE,),
                    min_val=0, max_val=32, skip_runtime_bounds_check=True,
                )
                li_w0, (w0_val,) = nc.values_load_multi_w_load_instructions(
                    scal[0:1, 16 * s + 5 : 16 * s + 6], engines=(DVE,),
                    min_val=0, max_val=128, skip_runtime_bounds_check=True,
                )
                for L in li_sh:
                    add_dep_helper(L.ins, w_sh.ins, sync=True, reason="reg load after s_h write")
                for L in li_w0:
                    add_dep_helper(L.ins, w_w0.ins, sync=True, reason="reg load after w0 write")
                out_sb = opool.tile([32, 1024], F32, tag="out_sb")
                o3 = out_sb[:, :].rearrange("p (a b) -> p a b", a=32)
                sel = d3[0:32, bass.ds(sh_val, 32), bass.ds(w0_val, 32)]
                nc.vector.tensor_copy(o3[:, :, :], sel)
                nc.sync.dma_start(y[32 * s : 32 * s + 32, :], out_sb[:, :])

          if loop_k is None:
              body()
          else:
              with tc.For_i(0, loop_k, 1) as iv:
                  body(iv)

    nc.compile()
    return nc


def get_nc(loop_k=None):
    key = ("nc", loop_k)
    if key not in _cache:
        _cache[key] = _build(loop_k)
    return _cache[key]


def kernel(x: np.ndarray, **run_kwargs) -> np.ndarray:
    assert x.shape == (32, 2, 64, 128, 128) and x.dtype == np.float32
    nc = get_nc()
    in_maps = []
    for c in range(N_CORES):
        xc = x[c * S_PER_CORE : (c + 1) * S_PER_CORE]           # [4, 2, 64, 128, 128]
        xc = np.ascontiguousarray(xc).reshape(N_VOLS, 128, FREE)
        in_maps.append({"x": xc})
    res = run_bass_kernel_spmd(nc, in_maps, core_ids=list(range(N_CORES)), **run_kwargs)
    out = np.empty((32, 2, 16, 32, 32), dtype=np.float32)
    for c in range(N_CORES):
        yc = res.results[c]["y"].reshape(S_PER_CORE, 2, 16, 32, 32)
        out[c * S_PER_CORE : (c + 1) * S_PER_CORE] = yc
    if run_kwargs:
        return out, res
    return out



# revision 12
# speedup vs baseline: 1.4543x; 1.4543x over previous
"""CenterCut2 Trainium2 kernel (v4 — bisection: baseline compute ops +
padded-window gather).

Same algorithm as v3 but every compute op uses the exact instruction mix
the proven baseline used: ACT square into power / in-place, DVE tensor_add,
MAX8 over the full power map, FIND_INDEX8, partition_broadcast(16) + 8
replication DMAs for the gather indices, and all DMAs issued from sync
except the gather (gpsimd).  The only novel mechanism left is dma_gather
with elem_step=192 over the padded copy.
"""
import sys

sys.path.insert(0, "/opt/trn_rl_repo")

import numpy as np

import concourse.bass as bass
import concourse.bacc as bacc
import concourse.mybir as mybir
from concourse.tile import TileContext
from concourse.tile_rust import add_dep_helper
from concourse.bass_utils import run_bass_kernel_spmd
from concourse.bass_isa import ReduceOp

import bass_rust

F32 = mybir.dt.float32
I32 = mybir.dt.int32
I16 = mybir.dt.int16
U32 = mybir.dt.uint32
A = mybir.AluOpType
DVE = mybir.EngineType.DVE

N_CORES = 8
S_PER_CORE = 4          # samples per core
N_VOLS = 2 * S_PER_CORE # channel volumes per core
VOL = 64 * 128 * 128    # voxels per volume
FREE = VOL // 128       # 8192 free elements per partition
CHUNK = 4096            # streaming chunk (2 MiB per DMA)
DP, HP, WPAD = 80, 160, 192
ROWS = DP * HP          # 12800 padded rows per volume
ESIZE = 32 * WPAD       # 6144-elem (24 KiB) gather run per (ch, dd)
BIG = float(1 << 21)

_cache = {}


def _build():
    nc = bacc.Bacc("TRN2", target_bir_lowering=False, debug=False, num_devices=N_CORES)
    x = nc.dram_tensor("x", [N_VOLS, 128, FREE], F32, kind="ExternalInput")
    xp = nc.dram_tensor("xpad", [N_VOLS, ROWS, WPAD], F32, kind="ExternalInput")
    y = nc.dram_tensor("y", [128, 1024], F32, kind="ExternalOutput")

    iota_base_c = nc.inline_tensor(
        (np.arange(128, dtype=np.float32) * FREE).reshape(128, 1), name="iota_base"
    )
    iotam_c = nc.inline_tensor(
        (np.arange(16, dtype=np.int32) * HP).reshape(16, 1), name="iotam"
    )

    with TileContext(nc) as tc:
        with (
            tc.tile_pool(name="xc", bufs=4) as xpool,
            tc.tile_pool(name="pw", bufs=2) as ppool,
            tc.tile_pool(name="gw", bufs=2) as gpool,
            tc.tile_pool(name="ob", bufs=2) as opool,
            tc.tile_pool(name="sm", bufs=2) as spool,
            tc.tile_pool(name="big", bufs=1) as bpool,
        ):
            base = bpool.tile([128, 1], F32, tag="base")
            nc.sync.dma_start(base[:, :], iota_base_c.ap()[:, :])
            iotam = bpool.tile([16, 1], I32, tag="iotam")
            nc.sync.dma_start(iotam[:, :], iotam_c.ap()[:, :])
            scal = bpool.tile([1, 64], I32, tag="scal")

            def ts1(dst, src, s1, op0):
                return nc.vector.tensor_scalar(
                    out=dst, in0=src, scalar1=s1, scalar2=None, op0=op0
                )

            gq = {}  # s -> (G tile, w_w0 writer) awaiting extract + output

            def emit_finish(s_):
                G_, w_w0 = gq.pop(s_)
                li_w, (w0v,) = nc.values_load_multi_w_load_instructions(
                    scal[:, 8 * s_ + 6 : 8 * s_ + 7], engines=(DVE,),
                    min_val=0, max_val=128, skip_runtime_bounds_check=True,
                )
                for L in li_w:
                    add_dep_helper(L.ins, w_w0.ins, sync=True, reason="reg load after w0 write")
                out_sb = opool.tile([32, 1024], F32, tag="ob")
                o3 = out_sb[:, :].rearrange("p (h w) -> p h w", w=32)
                G3 = G_[0:32, :].rearrange("p (h w) -> p h w", w=WPAD)
                nc.vector.tensor_copy(o3[:, :, :], G3[:, :, bass.ds(w0v, 32)])
                nc.sync.dma_start(y[32 * s_ : 32 * s_ + 32, :], out_sb[:, :])

            for s in range(S_PER_CORE):
                # ---- stream + power map (baseline op mix) ----
                power = ppool.tile([128, FREE], F32, tag="pw")
                for k in range(FREE // CHUNK):
                    sl = slice(k * CHUNK, (k + 1) * CHUNK)
                    x0 = xpool.tile([128, CHUNK], F32, tag="xc")
                    nc.sync.dma_start(x0[:, :], x[2 * s, :, sl])
                    nc.scalar.square(power[:, sl], x0[:, :])
                    x1 = xpool.tile([128, CHUNK], F32, tag="xc")
                    nc.sync.dma_start(x1[:, :], x[2 * s + 1, :, sl])
                    nc.scalar.square(x1[:, :], x1[:, :])
                    nc.vector.tensor_add(power[:, sl], power[:, sl], x1[:, :])

                if s - 1 in gq:
                    emit_finish(s - 1)

                # ---- global argmax with lowest-flat tie-break ----
                max8 = spool.tile([128, 8], F32, tag="mx")
                nc.vector.max(out=max8[:, :], in_=power[:, :])
                idx8 = spool.tile([128, 8], U32, tag="ix")
                nc.vector.max_index(out=idx8[:, :], in_max=max8[:, :], in_values=power[:, :])
                allmax = spool.tile([128, 1], F32, tag="am")
                nc.gpsimd.partition_all_reduce(allmax[:, :], max8[:, 0:1], 128, ReduceOp.max)

                flatf = spool.tile([128, 1], F32, tag="ff")
                nc.vector.tensor_copy(flatf[:, :], idx8[:, 0:1])      # uint32 -> f32
                nc.vector.tensor_add(flatf[:, :], flatf[:, :], base[:, :])
                eq = spool.tile([128, 1], F32, tag="eq")
                nc.vector.tensor_tensor(out=eq[:, :], in0=max8[:, 0:1], in1=allmax[:, :], op=A.is_equal)
                candneg = spool.tile([128, 1], F32, tag="cn")
                nc.vector.scalar_tensor_tensor(
                    out=candneg[:, :], in0=eq[:, :], scalar=BIG, in1=flatf[:, :],
                    op0=A.mult, op1=A.subtract,
                )
                allcand = spool.tile([128, 1], F32, tag="ac")
                nc.gpsimd.partition_all_reduce(allcand[:, :], candneg[:, :], 128, ReduceOp.max)

                # ---- decode flat -> d0, h0, w0, rowbase ----
                def C(j):
                    return scal[:, 8 * s + j : 8 * s + j + 1]

                flat32 = spool.tile([1, 1], F32, tag="f32")
                nc.vector.tensor_scalar(
                    out=flat32[:, :], in0=allcand[0:1, 0:1], scalar1=BIG, scalar2=-1.0,
                    op0=A.subtract, op1=A.mult,
                )
                nc.vector.tensor_copy(C(0), flat32[:, :])             # f32 -> int32
                ts1(C(1), C(0), 14, A.logical_shift_right)            # d
                nc.vector.tensor_scalar(
                    out=C(2), in0=C(0), scalar1=7, scalar2=127,
                    op0=A.logical_shift_right, op1=A.bitwise_and,
                )                                                     # h
                ts1(C(3), C(0), 127, A.bitwise_and)                   # w
                ts1(C(4), C(1), 56, A.add)
                ts1(C(4), C(4), 63, A.bitwise_and)                    # d0
                ts1(C(5), C(2), 112, A.add)
                ts1(C(5), C(5), 127, A.bitwise_and)                   # h0
                ts1(C(6), C(3), 112, A.add)
                w_w0 = ts1(C(6), C(6), 127, A.bitwise_and)            # w0
                # rowbase = d0*160 + h0  (d0*160 = d0<<7 + d0<<5)
                ts1(C(7), C(4), 7, A.logical_shift_left)
                ts1(C(1), C(4), 5, A.logical_shift_left)              # C1 (d) is dead
                nc.vector.tensor_tensor(out=C(7), in0=C(7), in1=C(1), op=A.add)
                nc.vector.tensor_tensor(out=C(7), in0=C(7), in1=C(5), op=A.add)

                # ---- gather rows (baseline-style 16-partition idx build) ----
                bc = spool.tile([16, 1], I32, tag="bc")
                nc.gpsimd.partition_broadcast(bc[:, :], C(7), channels=16)
                idx32 = spool.tile([16, 2], I32, tag="i32")
                nc.vector.tensor_tensor(out=idx32[:, 0:1], in0=iotam[:, :], in1=bc[:, :], op=A.add)
                ts1(idx32[:, 1:2], idx32[:, 0:1], ROWS, A.add)
                idx16 = spool.tile([16, 2], I16, tag="i16")
                nc.vector.tensor_copy(idx16[:, :], idx32[:, :])
                idxrep = spool.tile([128, 2], I16, tag="ir")
                for g in range(8):
                    nc.sync.dma_start(idxrep[16 * g : 16 * g + 16, :], idx16[:, :])

                G = gpool.tile([128, ESIZE], F32, tag="gw")
                src = xp.ap().copy()
                src.ap = bass_rust.VecI64Pair([[WPAD, 2 * ROWS - 32], [1, ESIZE]])
                src.offset = 2 * s * ROWS * WPAD
                nc.gpsimd.dma_gather(
                    out_ap=G[:, :].rearrange("p (a b) -> p a b", a=1),
                    in_ap=src,
                    idxs_ap=idxrep[:, :],
                    num_idxs=32,
                    num_idxs_reg=32,
                    elem_size=ESIZE,
                    elem_step=WPAD,
                )
                gq[s] = (G, w_w0)

            emit_finish(S_PER_CORE - 1)

    nc.compile()
    return nc


def get_nc():
    if "nc" not in _cache:
        _cache["nc"] = _build()
    return _cache["nc"]


def _pad_input(x: np.ndarray) -> np.ndarray:
    """Pad each (64,128,128) volume to (80,160,192): d,h circular by the
    window size; w circular to 160 then zero-filled to 192."""
    B, C = x.shape[0], x.shape[1]
    xpad = np.zeros((B, C, DP, HP, WPAD), dtype=np.float32)
    xpad[:, :, :64, :128, :128] = x
    xpad[:, :, 64:, :128, :128] = x[:, :, :16]
    xpad[:, :, :, 128:, :128] = xpad[:, :, :, :32, :128]
    xpad[:, :, :, :, 128:160] = xpad[:, :, :, :, :32]
    return xpad


def kernel(x: np.ndarray, **run_kwargs) -> np.ndarray:
    assert x.shape == (32, 2, 64, 128, 128) and x.dtype == np.float32
    nc = get_nc()
    xpad = _pad_input(x)
    in_maps = []
    for c in range(N_CORES):
        xc = x[c * S_PER_CORE : (c + 1) * S_PER_CORE]           # [4, 2, 64, 128, 128]
        xc = np.ascontiguousarray(xc).reshape(N_VOLS, 128, FREE)
        xpc = xpad[c * S_PER_CORE : (c + 1) * S_PER_CORE].reshape(N_VOLS, ROWS, WPAD)
        in_maps.append({"x": xc, "xpad": xpc})
    res = run_bass_kernel_spmd(nc, in_maps, core_ids=list(range(N_CORES)), **run_kwargs)
    out = np.empty((32, 2, 16, 32, 32), dtype=np.float32)
    for c in range(N_CORES):
        yc = res.results[c]["y"].reshape(S_PER_CORE, 2, 16, 32, 32)
        out[c * S_PER_CORE : (c + 1) * S_PER_CORE] = yc
    if run_kwargs:
        return out, res
    return out


# revision 20
# speedup vs baseline: 1.6200x; 1.1139x over previous
"""CenterCut2 Trainium2 kernel (v4 — bisection: baseline compute ops +
padded-window gather).

Same algorithm as v3 but every compute op uses the exact instruction mix
the proven baseline used: ACT square into power / in-place, DVE tensor_add,
MAX8 over the full power map, FIND_INDEX8, partition_broadcast(16) + 8
replication DMAs for the gather indices, and all DMAs issued from sync
except the gather (gpsimd).  The only novel mechanism left is dma_gather
with elem_step=192 over the padded copy.
"""
import sys

sys.path.insert(0, "/opt/trn_rl_repo")

import numpy as np

import concourse.bass as bass
import concourse.bacc as bacc
import concourse.mybir as mybir
from concourse.tile import TileContext
from concourse.tile_rust import add_dep_helper
from concourse.bass_utils import run_bass_kernel_spmd
from concourse.bass_isa import ReduceOp

import bass_rust

F32 = mybir.dt.float32
I32 = mybir.dt.int32
I16 = mybir.dt.int16
U32 = mybir.dt.uint32
A = mybir.AluOpType
DVE = mybir.EngineType.DVE

N_CORES = 8
S_PER_CORE = 4          # samples per core
N_VOLS = 2 * S_PER_CORE # channel volumes per core
VOL = 64 * 128 * 128    # voxels per volume
FREE = VOL // 128       # 8192 free elements per partition
CHUNK = 4096            # streaming chunk (2 MiB per DMA)
DP, HP, WPAD = 80, 160, 192
ROWS = DP * HP          # 12800 padded rows per volume
ESIZE = 32 * WPAD       # 6144-elem (24 KiB) gather run per (ch, dd)
BIG = float(1 << 21)

_cache = {}


def _build():
    nc = bacc.Bacc("TRN2", target_bir_lowering=False, debug=False, num_devices=N_CORES)
    x = nc.dram_tensor("x", [N_VOLS, 128, FREE], F32, kind="ExternalInput")
    xp = nc.dram_tensor("xpad", [N_VOLS, ROWS, WPAD], F32, kind="ExternalInput")
    y = nc.dram_tensor("y", [128, 1024], F32, kind="ExternalOutput")

    iota_base_c = nc.inline_tensor(
        (np.arange(128, dtype=np.float32) * FREE).reshape(128, 1), name="iota_base"
    )
    iotam_c = nc.inline_tensor(
        np.tile(np.arange(16, dtype=np.int32) * HP, 8).reshape(128, 1), name="iotam"
    )

    with TileContext(nc) as tc:
        with (
            tc.tile_pool(name="xc", bufs=4) as xpool,
            tc.tile_pool(name="pw", bufs=2) as ppool,
            tc.tile_pool(name="gw", bufs=2) as gpool,
            tc.tile_pool(name="ob", bufs=2) as opool,
            tc.tile_pool(name="sm", bufs=2) as spool,
            tc.tile_pool(name="big", bufs=1) as bpool,
        ):
            base = bpool.tile([128, 1], F32, tag="base")
            nc.sync.dma_start(base[:, :], iota_base_c.ap()[:, :])
            iotam = bpool.tile([128, 1], I32, tag="iotam")
            nc.sync.dma_start(iotam[:, :], iotam_c.ap()[:, :])
            scal = bpool.tile([1, 64], I32, tag="scal")

            def ts1(dst, src, s1, op0):
                return nc.vector.tensor_scalar(
                    out=dst, in0=src, scalar1=s1, scalar2=None, op0=op0
                )

            gq = {}  # s -> (G tile, w_w0 writer) awaiting extract + output

            def emit_finish(s_):
                G_, w_w0 = gq.pop(s_)
                li_w, (w0v,) = nc.values_load_multi_w_load_instructions(
                    scal[:, 8 * s_ + 6 : 8 * s_ + 7], engines=(DVE,),
                    min_val=0, max_val=128, skip_runtime_bounds_check=True,
                )
                for L in li_w:
                    add_dep_helper(L.ins, w_w0.ins, sync=True, reason="reg load after w0 write")
                out_sb = opool.tile([32, 1024], F32, tag="ob")
                o3 = out_sb[:, :].rearrange("p (h w) -> p h w", w=32)
                G3 = G_[0:32, :].rearrange("p (h w) -> p h w", w=WPAD)
                nc.vector.tensor_copy(o3[:, :, :], G3[:, :, bass.ds(w0v, 32)])
                nc.sync.dma_start(y[32 * s_ : 32 * s_ + 32, :], out_sb[:, :])

            for s in range(S_PER_CORE):
                # ---- stream + power map (baseline op mix) ----
                power = ppool.tile([128, FREE], F32, tag="pw")
                for k in range(FREE // CHUNK):
                    sl = slice(k * CHUNK, (k + 1) * CHUNK)
                    x0 = xpool.tile([128, CHUNK], F32, tag="xc")
                    nc.sync.dma_start(x0[:, :], x[2 * s, :, sl])
                    nc.scalar.square(power[:, sl], x0[:, :])
                    x1 = xpool.tile([128, CHUNK], F32, tag="xc")
                    nc.sync.dma_start(x1[:, :], x[2 * s + 1, :, sl])
                    nc.scalar.square(x1[:, :], x1[:, :])
                    nc.vector.tensor_add(power[:, sl], power[:, sl], x1[:, :])

                if s - 1 in gq:
                    emit_finish(s - 1)

                # ---- global argmax with lowest-flat tie-break ----
                max8 = spool.tile([128, 8], F32, tag="mx")
                nc.vector.max(out=max8[:, :], in_=power[:, :])
                idx8 = spool.tile([128, 8], U32, tag="ix")
                nc.vector.max_index(out=idx8[:, :], in_max=max8[:, :], in_values=power[:, :])
                allmax = spool.tile([128, 1], F32, tag="am")
                nc.gpsimd.partition_all_reduce(allmax[:, :], max8[:, 0:1], 128, ReduceOp.max)

                flatf = spool.tile([128, 1], F32, tag="ff")
                nc.vector.tensor_copy(flatf[:, :], idx8[:, 0:1])      # uint32 -> f32
                nc.vector.tensor_add(flatf[:, :], flatf[:, :], base[:, :])
                eq = spool.tile([128, 1], F32, tag="eq")
                nc.vector.tensor_tensor(out=eq[:, :], in0=max8[:, 0:1], in1=allmax[:, :], op=A.is_equal)
                candneg = spool.tile([128, 1], F32, tag="cn")
                nc.vector.scalar_tensor_tensor(
                    out=candneg[:, :], in0=eq[:, :], scalar=BIG, in1=flatf[:, :],
                    op0=A.mult, op1=A.subtract,
                )
                allcand = spool.tile([128, 1], F32, tag="ac")
                nc.gpsimd.partition_all_reduce(allcand[:, :], candneg[:, :], 128, ReduceOp.max)

                # ---- decode flat -> d0, h0, w0, rowbase ----
                def C(j):
                    return scal[:, 8 * s + j : 8 * s + j + 1]

                flat32 = spool.tile([1, 1], F32, tag="f32")
                nc.vector.tensor_scalar(
                    out=flat32[:, :], in0=allcand[0:1, 0:1], scalar1=BIG, scalar2=-1.0,
                    op0=A.subtract, op1=A.mult,
                )
                nc.vector.tensor_copy(C(0), flat32[:, :])             # f32 -> int32
                ts1(C(1), C(0), 14, A.logical_shift_right)            # d
                nc.vector.tensor_scalar(
                    out=C(2), in0=C(0), scalar1=7, scalar2=127,
                    op0=A.logical_shift_right, op1=A.bitwise_and,
                )                                                     # h
                ts1(C(3), C(0), 127, A.bitwise_and)                   # w
                ts1(C(4), C(1), 56, A.add)
                ts1(C(4), C(4), 63, A.bitwise_and)                    # d0
                ts1(C(5), C(2), 112, A.add)
                ts1(C(5), C(5), 127, A.bitwise_and)                   # h0
                ts1(C(6), C(3), 112, A.add)
                w_w0 = ts1(C(6), C(6), 127, A.bitwise_and)            # w0
                # rowbase = d0*160 + h0  (d0*160 = d0<<7 + d0<<5)
                ts1(C(7), C(4), 7, A.logical_shift_left)
                ts1(C(1), C(4), 5, A.logical_shift_left)              # C1 (d) is dead
                nc.vector.tensor_tensor(out=C(7), in0=C(7), in1=C(1), op=A.add)
                nc.vector.tensor_tensor(out=C(7), in0=C(7), in1=C(5), op=A.add)

                # ---- gather rows: idx[p, c] = (p%16)*160 + rowbase + c*12800 ----
                bc = spool.tile([128, 1], I32, tag="bc")
                nc.gpsimd.partition_broadcast(bc[:, :], C(7), channels=128)
                idx32 = spool.tile([128, 2], I32, tag="i32")
                nc.vector.tensor_tensor(out=idx32[:, 0:1], in0=iotam[:, :], in1=bc[:, :], op=A.add)
                ts1(idx32[:, 1:2], idx32[:, 0:1], ROWS, A.add)
                idxrep = spool.tile([128, 2], I16, tag="ir")
                nc.vector.tensor_copy(idxrep[:, :], idx32[:, :])

                G = gpool.tile([128, ESIZE], F32, tag="gw")
                src = xp.ap().copy()
                src.ap = bass_rust.VecI64Pair([[WPAD, 2 * ROWS - 32], [1, ESIZE]])
                src.offset = 2 * s * ROWS * WPAD
                nc.gpsimd.dma_gather(
                    out_ap=G[:, :].rearrange("p (a b) -> p a b", a=1),
                    in_ap=src,
                    idxs_ap=idxrep[:, :],
                    num_idxs=32,
                    num_idxs_reg=32,
                    elem_size=ESIZE,
                    elem_step=WPAD,
                )
                gq[s] = (G, w_w0)

            emit_finish(S_PER_CORE - 1)

    nc.compile()
    return nc


def get_nc():
    if "nc" not in _cache:
        _cache["nc"] = _build()
    return _cache["nc"]


def _pad_input(x: np.ndarray) -> np.ndarray:
    """Pad each (64,128,128) volume to (80,160,192): d,h circular by the
    window size; w circular to 160 then zero-filled to 192."""
    B, C = x.shape[0], x.shape[1]
    xpad = np.zeros((B, C, DP, HP, WPAD), dtype=np.float32)
    xpad[:, :, :64, :128, :128] = x
    xpad[:, :, 64:, :128, :128] = x[:, :, :16]
    xpad[:, :, :, 128:, :128] = xpad[:, :, :, :32, :128]
    xpad[:, :, :, :, 128:160] = xpad[:, :, :, :, :32]
    return xpad


def kernel(x: np.ndarray, **run_kwargs) -> np.ndarray:
    assert x.shape == (32, 2, 64, 128, 128) and x.dtype == np.float32
    nc = get_nc()
    xpad = _pad_input(x)
    in_maps = []
    for c in range(N_CORES):
        xc = x[c * S_PER_CORE : (c + 1) * S_PER_CORE]           # [4, 2, 64, 128, 128]
        xc = np.ascontiguousarray(xc).reshape(N_VOLS, 128, FREE)
        xpc = xpad[c * S_PER_CORE : (c + 1) * S_PER_CORE].reshape(N_VOLS, ROWS, WPAD)
        in_maps.append({"x": xc, "xpad": xpc})
    res = run_bass_kernel_spmd(nc, in_maps, core_ids=list(range(N_CORES)), **run_kwargs)
    out = np.empty((32, 2, 16, 32, 32), dtype=np.float32)
    for c in range(N_CORES):
        yc = res.results[c]["y"].reshape(S_PER_CORE, 2, 16, 32, 32)
        out[c * S_PER_CORE : (c + 1) * S_PER_CORE] = yc
    if run_kwargs:
        return out, res
    return out
